# revision 15
# baseline (speedup 1.0000x reference)
"""DPI neuron forward step on 8 Trainium2 cores (raw Bass, explicit sems).

Math (forward only, f32):
  Sa = X @ round(W_ampa).T ; Ss = X @ round(W_shunt).T
  A = Iw_a*Sa ; S = Iw_s*Ss
  Iin  = max((Ia0 + A - Is0 - S + c_in) * (rf0<=0), I0)
  Ifb  = (C0/2) * Im0^p          (sigmoid denom == 2.0 exactly in f32)
  f    = Ifb * (1e12*Im0 + 1)
  num  = Iin - c2 - 1.05*Im0 + f
  G    = (dt/taum) * Im0/(Im0+1e-12)
  Im1  = max(Im0 + num*G, I0)
  spike = Im1 > Ith ; Imem = spike? I0 : Im1
  Iampa = max(k1*Ia0 + A, I0) ; Ishunt = max(k1*Is0 + S, I0)
  refr  = spike? 0 : max(rf0-dt, 0)

Sharding: batch 4096 -> 8 cores x 512 rows; weights replicated (bf16 is
exact for round(W) in 0..4). On-device layout is TRANSPOSED [n_out, b]:
psum[o=128, b=512] = W_tile[k,o].T @ XT[k,b], W tiles streamed as the
stationary operand (host pre-tiled contiguous), X.T resident moving.
16 o_tile iterations, 2-deep double-buffered pipeline across
SP(dma-in) / PE / ACT(+dma-out) / GPSIMD / DVE with explicit semaphores.
"""

import math
import numpy as np
import ml_dtypes

bf16 = ml_dtypes.bfloat16

B, N_IN, N_OUT = 4096, 2048, 2048
NCORES = 8
BL = B // NCORES          # 512 batch rows per core
OT = N_OUT // 128         # 16 o_tiles
KT = N_IN // 128          # 16 contraction tiles

# model constants
I0 = 5e-14
kappa = (0.75 + 0.66) / 2
dt = 1e-3
tau_mem = 0.025 / kappa * 3.0
tau_ampa = 0.025 / kappa * 2.0
tau_shunt = 0.025 / kappa * 2.0
Ith = 1e-12
p_pow = kappa / (kappa + 1.0)
lnC0h = (1.0 / (kappa + 1.0)) * math.log(I0) - math.log(2.0)
c_in = 1e-12 + I0
c2 = 1e-12 + I0
k1a = 1.0 - dt / tau_ampa
k1s = 1.0 - dt / tau_shunt
G_scale = -(1e-12) * dt / tau_mem
G_bias = dt / tau_mem

_CACHE = {}
LAST_RESULT = None


def _build(ca: float, cs: float):
    import concourse.bass as bass
    from concourse import mybir

    f32 = mybir.dt.float32
    b16 = mybir.dt.bfloat16
    u8 = mybir.dt.uint8
    Alu = mybir.AluOpType
    Act = mybir.ActivationFunctionType

    nc = bass.Bass()
    xt_d = nc.declare_dram_parameter("xt", [N_IN, BL], b16, isOutput=False)
    wa_d = nc.declare_dram_parameter("wat", [OT, KT, 128, 128], b16,
                                     isOutput=False)
    ws_d = nc.declare_dram_parameter("wst", [OT, KT, 128, 128], b16,
                                     isOutput=False)
    im_d = nc.declare_dram_parameter("im0", [N_OUT, BL], f32, isOutput=False)
    ia_d = nc.declare_dram_parameter("ia0", [N_OUT, BL], b16, isOutput=False)
    is_d = nc.declare_dram_parameter("is0", [N_OUT, BL], b16, isOutput=False)
    rf_d = nc.declare_dram_parameter("rf0", [N_OUT, BL], b16, isOutput=False)
    spk_o = nc.declare_dram_parameter("spk", [N_OUT, BL], b16, isOutput=True)
    imem_o = nc.declare_dram_parameter("imem", [N_OUT, BL], f32, isOutput=True)
    iamp_o = nc.declare_dram_parameter("iamp", [N_OUT, BL], f32, isOutput=True)
    ishu_o = nc.declare_dram_parameter("ishu", [N_OUT, BL], f32, isOutput=True)
    refr_o = nc.declare_dram_parameter("refr", [N_OUT, BL], b16, isOutput=True)

    sb = nc.alloc_sbuf_tensor
    ps = nc.alloc_psum_tensor
    xt_sb = [sb(f"xt{k}", [128, BL], b16) for k in range(KT)]
    wa_b = [sb(f"wa{p}", [128, KT * 128], b16) for p in range(2)]
    ws_b = [sb(f"ws{p}", [128, KT * 128], b16) for p in range(2)]
    im0_b = [sb(f"im0{p}", [128, BL], f32) for p in range(2)]
    ia0_b = [sb(f"ia0{p}", [128, BL], b16) for p in range(2)]
    is0_b = [sb(f"is0{p}", [128, BL], b16) for p in range(2)]
    rf0_b = [sb(f"rf0{p}", [128, BL], b16) for p in range(2)]
    imem_b = [sb(f"imem{p}", [128, BL], f32) for p in range(2)]
    iamp_b = [sb(f"iamp{p}", [128, BL], f32) for p in range(2)]
    ishu_b = [sb(f"ishu{p}", [128, BL], f32) for p in range(2)]
    spk_b = [sb(f"spk{p}", [128, BL], b16) for p in range(2)]
    refr_b = [sb(f"refr{p}", [128, BL], b16) for p in range(2)]
    names = "A S lnim v w l2 r2 G ua us m e d1 q qm iin f n1 num dimdt".split()
    T = {nm: [sb(f"{nm}{p}", [128, BL], f32) for p in range(2)] for nm in names}
    spkm_b = [sb(f"spkm{p}", [128, BL], u8) for p in range(2)]
    v1_b = [sb(f"v1{p}", [128, BL], b16) for p in range(2)]
    i0t = sb("i0t", [128, BL], f32)
    zbt = sb("zbt", [128, BL], b16)
    lnC0h_t = sb("lnC0h_t", [128, 1], f32)
    eps_t = sb("eps_t", [128, 1], f32)
    zero_t = sb("zero_t", [128, 1], f32)
    psa_b = [ps(f"psa{p}", [128, BL], f32) for p in range(2)]
    pss_b = [ps(f"pss{p}", [128, BL], f32) for p in range(2)]

    with (
        nc.Block() as block,
        nc.semaphore("x_sem") as x_sem,
        nc.semaphore("w_sem") as w_sem,
        nc.semaphore("st_sem") as st_sem,
        nc.semaphore("mm_sem") as mm_sem,
        nc.semaphore("evac_sem") as evac_sem,
        nc.semaphore("act_sem") as act_sem,
        nc.semaphore("gp_sem") as gp_sem,
        nc.semaphore("gp2_sem") as gp2_sem,
        nc.semaphore("dvs_sem") as dvs_sem,
        nc.semaphore("dv_sem") as dv_sem,
        nc.semaphore("out_sem") as out_sem,
        nc.semaphore("cst_sem") as cst_sem,
    ):
        @block.sync
        def _(sync):
            for k in range(KT):
                sync.dma_start(out=xt_sb[k].ap(),
                               in_=xt_d[k * 128:(k + 1) * 128, :]
                               ).then_inc(x_sem, 16)
            for i in range(OT):
                p = i % 2
                if i >= 2:
                    sync.wait_ge(mm_sem, i - 1)
                sync.dma_start(
                    out=wa_b[p].ap().rearrange("r (k c) -> r k c", k=KT),
                    in_=wa_d[i].rearrange("k r c -> r k c")
                ).then_inc(w_sem, 16)
                sync.dma_start(
                    out=ws_b[p].ap().rearrange("r (k c) -> r k c", k=KT),
                    in_=ws_d[i].rearrange("k r c -> r k c")
                ).then_inc(w_sem, 16)
                if i >= 2:
                    sync.wait_ge(dv_sem, i - 1)
                rs = slice(i * 128, (i + 1) * 128)
                sync.dma_start(out=im0_b[p].ap(), in_=im_d[rs, :]
                               ).then_inc(st_sem, 16)
                sync.dma_start(out=ia0_b[p].ap(), in_=ia_d[rs, :]
                               ).then_inc(st_sem, 16)
                sync.dma_start(out=is0_b[p].ap(), in_=is_d[rs, :]
                               ).then_inc(st_sem, 16)
                sync.dma_start(out=rf0_b[p].ap(), in_=rf_d[rs, :]
                               ).then_inc(st_sem, 16)

        @block.tensor
        def _(pe):
            pe.wait_ge(x_sem, 16 * KT)
            for i in range(OT):
                p = i % 2
                pe.wait_ge(w_sem, 32 * (i + 1))
                if i >= 2:
                    pe.wait_ge(evac_sem, i - 1)
                last = None
                for k in range(KT):
                    st, sp = (k == 0), (k == KT - 1)
                    pe.matmul(psa_b[p].ap(),
                              wa_b[p].ap()[:, k * 128:(k + 1) * 128],
                              xt_sb[k].ap(), start=st, stop=sp)
                    last = pe.matmul(pss_b[p].ap(),
                                     ws_b[p].ap()[:, k * 128:(k + 1) * 128],
                                     xt_sb[k].ap(), start=st, stop=sp)
                last.then_inc(mm_sem, 1)

        @block.scalar
        def _(act):
            act.wait_ge(cst_sem, 1)
            for i in range(OT):
                p = i % 2
                act.wait_ge(mm_sem, i + 1)
                act.wait_ge(st_sem, 64 * (i + 1))
                if i >= 2:
                    act.wait_ge(dv_sem, i - 1)
                act.activation(T["A"][p].ap(), psa_b[p].ap(), Act.Copy,
                               scale=ca)
                act.activation(T["S"][p].ap(), pss_b[p].ap(), Act.Copy,
                               scale=cs).then_inc(evac_sem, 1)
                act.activation(T["lnim"][p].ap(), im0_b[p].ap(), Act.Ln,
                               bias=zero_t.ap())
                act.activation(T["v"][p].ap(), T["lnim"][p].ap(), Act.Exp,
                               scale=p_pow, bias=lnC0h_t.ap())
                act.activation(T["w"][p].ap(), im0_b[p].ap(), Act.Copy,
                               scale=1e12, bias=1.0)
                act.activation(T["l2"][p].ap(), im0_b[p].ap(), Act.Ln,
                               bias=eps_t.ap())
                act.activation(T["r2"][p].ap(), T["l2"][p].ap(), Act.Exp,
                               scale=-1.0, bias=zero_t.ap())
                act.activation(T["G"][p].ap(), T["r2"][p].ap(), Act.Copy,
                               scale=G_scale, bias=G_bias
                               ).then_inc(act_sem, 1)
                if i >= 1:
                    q_ = (i - 1) % 2
                    o_rs = slice((i - 1) * 128, i * 128)
                    act.wait_ge(dv_sem, i)
                    act.wait_ge(gp2_sem, i)
                    act.dma_start(out=imem_o[o_rs, :], in_=imem_b[q_].ap()
                                  ).then_inc(out_sem, 16)
                    act.dma_start(out=iamp_o[o_rs, :], in_=iamp_b[q_].ap()
                                  ).then_inc(out_sem, 16)
                    act.dma_start(out=ishu_o[o_rs, :], in_=ishu_b[q_].ap()
                                  ).then_inc(out_sem, 16)
                    act.dma_start(out=spk_o[o_rs, :], in_=spk_b[q_].ap()
                                  ).then_inc(out_sem, 16)
                    act.dma_start(out=refr_o[o_rs, :], in_=refr_b[q_].ap()
                                  ).then_inc(out_sem, 16)
            act.wait_ge(dv_sem, OT)
            act.wait_ge(gp2_sem, OT)
            q_ = (OT - 1) % 2
            o_rs = slice((OT - 1) * 128, OT * 128)
            act.dma_start(out=imem_o[o_rs, :], in_=imem_b[q_].ap()
                          ).then_inc(out_sem, 16)
            act.dma_start(out=iamp_o[o_rs, :], in_=iamp_b[q_].ap()
                          ).then_inc(out_sem, 16)
            act.dma_start(out=ishu_o[o_rs, :], in_=ishu_b[q_].ap()
                          ).then_inc(out_sem, 16)
            act.dma_start(out=spk_o[o_rs, :], in_=spk_b[q_].ap()
                          ).then_inc(out_sem, 16)
            act.dma_start(out=refr_o[o_rs, :], in_=refr_b[q_].ap()
                          ).then_inc(out_sem, 16)

        @block.gpsimd
        def _(gp):
            for i in range(OT):
                p = i % 2
                gp.wait_ge(act_sem, i + 1)
                if i >= 2:
                    gp.wait_ge(out_sem, 80 * (i - 1))
                gp.tensor_scalar(T["ua"][p].ap(), ia0_b[p].ap(), k1a, None,
                                 Alu.mult)
                gp.tensor_tensor(T["ua"][p].ap(), T["ua"][p].ap(),
                                 T["A"][p].ap(), Alu.add)
                gp.tensor_scalar(iamp_b[p].ap(), T["ua"][p].ap(), I0, None,
                                 Alu.max)
                gp.tensor_scalar(T["us"][p].ap(), is0_b[p].ap(), k1s, None,
                                 Alu.mult)
                gp.tensor_tensor(T["us"][p].ap(), T["us"][p].ap(),
                                 T["S"][p].ap(), Alu.add)
                gp.tensor_scalar(ishu_b[p].ap(), T["us"][p].ap(), I0, None,
                                 Alu.max)
                gp.tensor_scalar(T["m"][p].ap(), rf0_b[p].ap(), 0.0, None,
                                 Alu.is_le)
                gp.tensor_scalar(T["e"][p].ap(), ia0_b[p].ap(), c_in, None,
                                 Alu.add)
                gp.tensor_tensor(T["e"][p].ap(), T["e"][p].ap(),
                                 is0_b[p].ap(), Alu.subtract)
                gp.tensor_scalar(v1_b[p].ap(), rf0_b[p].ap(), -dt, 0.0,
                                 Alu.add, Alu.max).then_inc(gp_sem, 1)
                gp.wait_ge(dvs_sem, i + 1)
                gp.tensor_copy(spk_b[p].ap(), spkm_b[p].ap()
                               ).then_inc(gp2_sem, 1)

        @block.vector
        def _(dv):
            dv.memset(i0t.ap(), I0)
            dv.memset(zbt.ap(), 0.0)
            dv.memset(lnC0h_t.ap(), lnC0h)
            dv.memset(eps_t.ap(), 1e-12)
            dv.memset(zero_t.ap(), 0.0).then_inc(cst_sem, 1)
            for i in range(OT):
                p = i % 2
                dv.wait_ge(gp_sem, i + 1)
                if i >= 2:
                    dv.wait_ge(out_sem, 80 * (i - 1))
                dv.tensor_tensor(T["d1"][p].ap(), T["A"][p].ap(),
                                 T["S"][p].ap(), Alu.subtract)
                dv.tensor_tensor(T["q"][p].ap(), T["e"][p].ap(),
                                 T["d1"][p].ap(), Alu.add)
                dv.tensor_tensor(T["qm"][p].ap(), T["q"][p].ap(),
                                 T["m"][p].ap(), Alu.mult)
                dv.tensor_scalar(T["iin"][p].ap(), T["qm"][p].ap(), I0, None,
                                 Alu.max)
                dv.tensor_tensor(T["f"][p].ap(), T["v"][p].ap(),
                                 T["w"][p].ap(), Alu.mult)
                dv.scalar_tensor_tensor(T["n1"][p].ap(), im0_b[p].ap(), -1.05,
                                        T["f"][p].ap(), Alu.mult, Alu.add)
                dv.scalar_tensor_tensor(T["num"][p].ap(), T["n1"][p].ap(),
                                        -c2, T["iin"][p].ap(), Alu.add,
                                        Alu.add)
                dv.tensor_tensor(T["dimdt"][p].ap(), T["num"][p].ap(),
                                 T["G"][p].ap(), Alu.mult)
                dv.tensor_tensor(imem_b[p].ap(), im0_b[p].ap(),
                                 T["dimdt"][p].ap(), Alu.add)
                dv.tensor_scalar(imem_b[p].ap(), imem_b[p].ap(), I0, None,
                                 Alu.max)
                dv.tensor_scalar(spkm_b[p].ap(), imem_b[p].ap(), Ith, None,
                                 Alu.is_gt).then_inc(dvs_sem, 1)
                dv.copy_predicated(imem_b[p].ap(), spkm_b[p].ap(), i0t.ap())
                dv.tensor_copy(refr_b[p].ap(), v1_b[p].ap())
                dv.copy_predicated(refr_b[p].ap(), spkm_b[p].ap(), zbt.ap()
                                   ).then_inc(dv_sem, 1)

    nc.finalize()
    return nc


def _tile_w(wt):
    # [N_IN, N_OUT] -> [OT, KT, 128, 128] contiguous (ot, k, k128, o128)
    return np.ascontiguousarray(
        wt.reshape(KT, 128, OT, 128).transpose(2, 0, 1, 3))


def _prep_in_maps(X, W_ampa, W_shunt, Imem, Iampa, Ishunt, refractory):
    XT = np.ascontiguousarray(np.asarray(X, np.float32).T).astype(bf16)
    WaT = _tile_w(np.round(np.asarray(W_ampa, np.float32)).T.astype(bf16))
    WsT = _tile_w(np.round(np.asarray(W_shunt, np.float32)).T.astype(bf16))
    ImT = np.ascontiguousarray(np.asarray(Imem, np.float32).T)
    IaT = np.ascontiguousarray(np.asarray(Iampa, np.float32).T).astype(bf16)
    IsT = np.ascontiguousarray(np.asarray(Ishunt, np.float32).T).astype(bf16)
    RfT = np.ascontiguousarray(np.asarray(refractory, np.float32).T
                               ).astype(bf16)

    in_maps = []
    for c in range(NCORES):
        cs_ = slice(c * BL, (c + 1) * BL)
        in_maps.append(dict(
            xt=np.ascontiguousarray(XT[:, cs_]),
            wat=WaT, wst=WsT,
            im0=np.ascontiguousarray(ImT[:, cs_]),
            ia0=np.ascontiguousarray(IaT[:, cs_]),
            is0=np.ascontiguousarray(IsT[:, cs_]),
            rf0=np.ascontiguousarray(RfT[:, cs_]),
        ))
    return in_maps


def kernel(X, W_ampa, W_shunt, Iw_ampa, Iw_shunt, Imem, Iampa, Ishunt,
           refractory):
    from concourse.bass_utils import run_bass_kernel_spmd

    ca = float(np.float32(Iw_ampa))          # Igain_ampa/Itau_ampa == 1
    cs = float(np.float32(Iw_shunt))
    key = (ca, cs)
    if key not in _CACHE:
        _CACHE[key] = _build(ca, cs)
    nc = _CACHE[key]

    in_maps = _prep_in_maps(X, W_ampa, W_shunt, Imem, Iampa, Ishunt,
                            refractory)
    global LAST_RESULT
    r = run_bass_kernel_spmd(nc, in_maps, list(range(NCORES)))
    LAST_RESULT = r
    res = r.results

    def gather(name, cast=False):
        full = np.concatenate([r[name] for r in res], 1)   # [N_OUT, B]
        out = np.ascontiguousarray(full.T)
        return out.astype(np.float32) if cast else out

    spike = gather("spk", cast=True)
    imem = gather("imem")
    iampa = gather("iamp")
    ishunt = gather("ishu")
    refr = gather("refr", cast=True)
    return spike, (imem, iampa, ishunt, refr)


# revision 16
# speedup vs baseline: 4.5982x; 4.5982x over previous
"""DPI neuron forward step on 8 Trainium2 cores (raw Bass, explicit sems).

Math (forward only, f32):
  Sa = X @ round(W_ampa).T ; Ss = X @ round(W_shunt).T
  A = Iw_a*Sa ; S = Iw_s*Ss
  Iin  = max((Ia0 + A - Is0 - S + c_in) * (rf0<=0), I0)
  Ifb  = (C0/2) * Im0^p          (sigmoid denom == 2.0 exactly in f32)
  f    = Ifb * (1e12*Im0 + 1)
  num  = Iin - c2 - 1.05*Im0 + f
  G    = (dt/taum) * Im0/(Im0+1e-12)
  Im1  = max(Im0 + num*G, I0)
  spike = Im1 > Ith ; Imem = spike? I0 : Im1
  Iampa = max(k1*Ia0 + A, I0) ; Ishunt = max(k1*Is0 + S, I0)
  refr  = spike? 0 : max(rf0-dt, 0)

Sharding: batch 4096 -> 8 cores x 512 rows; weights replicated (bf16 is
exact for round(W) in 0..4). On-device layout is TRANSPOSED [n_out, b]:
psum[o=128, b=512] = W_tile[k,o].T @ XT[k,b], W tiles streamed as the
stationary operand (host pre-tiled contiguous), X.T resident moving.
16 o_tile iterations, 2-deep double-buffered pipeline across
SP(dma-in) / PE / ACT(+dma-out) / GPSIMD / DVE with explicit semaphores.
"""

import math
import numpy as np
import ml_dtypes

bf16 = ml_dtypes.bfloat16

B, N_IN, N_OUT = 4096, 2048, 2048
NCORES = 8
BL = B // NCORES          # 512 batch rows per core
OT = N_OUT // 128         # 16 o_tiles
KT = N_IN // 128          # 16 contraction tiles

# model constants
I0 = 5e-14
kappa = (0.75 + 0.66) / 2
dt = 1e-3
tau_mem = 0.025 / kappa * 3.0
tau_ampa = 0.025 / kappa * 2.0
tau_shunt = 0.025 / kappa * 2.0
Ith = 1e-12
p_pow = kappa / (kappa + 1.0)
lnC0h = (1.0 / (kappa + 1.0)) * math.log(I0) - math.log(2.0)
c_in = 1e-12 + I0
c2 = 1e-12 + I0
k1a = 1.0 - dt / tau_ampa
k1s = 1.0 - dt / tau_shunt
G_scale = -(1e-12) * dt / tau_mem
G_bias = dt / tau_mem

_CACHE = {}
LAST_RESULT = None


def _build(ca: float, cs: float, repeat: int = 1):
    import concourse.bass as bass
    from concourse import mybir

    f32 = mybir.dt.float32
    b16 = mybir.dt.bfloat16
    u8 = mybir.dt.uint8
    Alu = mybir.AluOpType
    Act = mybir.ActivationFunctionType

    nc = bass.Bass()
    xt_d = nc.declare_dram_parameter("xt", [N_IN, BL], b16, isOutput=False)
    wa_d = nc.declare_dram_parameter("wat", [OT, KT, 128, 128], b16,
                                     isOutput=False)
    ws_d = nc.declare_dram_parameter("wst", [OT, KT, 128, 128], b16,
                                     isOutput=False)
    im_d = nc.declare_dram_parameter("im0", [N_OUT, BL], f32, isOutput=False)
    ia_d = nc.declare_dram_parameter("ia0", [N_OUT, BL], b16, isOutput=False)
    is_d = nc.declare_dram_parameter("is0", [N_OUT, BL], b16, isOutput=False)
    rf_d = nc.declare_dram_parameter("rf0", [N_OUT, BL], b16, isOutput=False)
    spk_o = nc.declare_dram_parameter("spk", [N_OUT, BL], b16, isOutput=True)
    imem_o = nc.declare_dram_parameter("imem", [N_OUT, BL], f32, isOutput=True)
    iamp_o = nc.declare_dram_parameter("iamp", [N_OUT, BL], f32, isOutput=True)
    ishu_o = nc.declare_dram_parameter("ishu", [N_OUT, BL], f32, isOutput=True)
    refr_o = nc.declare_dram_parameter("refr", [N_OUT, BL], b16, isOutput=True)

    sb = nc.alloc_sbuf_tensor
    ps = nc.alloc_psum_tensor
    xt_sb = [sb(f"xt{k}", [128, BL], b16) for k in range(KT)]
    wa_b = [sb(f"wa{p}", [128, KT * 128], b16) for p in range(2)]
    ws_b = [sb(f"ws{p}", [128, KT * 128], b16) for p in range(2)]
    im0_b = [sb(f"im0{p}", [128, BL], f32) for p in range(2)]
    ia0_b = [sb(f"ia0{p}", [128, BL], b16) for p in range(2)]
    is0_b = [sb(f"is0{p}", [128, BL], b16) for p in range(2)]
    rf0_b = [sb(f"rf0{p}", [128, BL], b16) for p in range(2)]
    imem_b = [sb(f"imem{p}", [128, BL], f32) for p in range(2)]
    iamp_b = [sb(f"iamp{p}", [128, BL], f32) for p in range(2)]
    ishu_b = [sb(f"ishu{p}", [128, BL], f32) for p in range(2)]
    spk_b = [sb(f"spk{p}", [128, BL], b16) for p in range(2)]
    refr_b = [sb(f"refr{p}", [128, BL], b16) for p in range(2)]
    names = "A S lnim v w l2 r2 G ua us m e d1 q qm iin f n1 num dimdt".split()
    T = {nm: [sb(f"{nm}{p}", [128, BL], f32) for p in range(2)] for nm in names}
    spkm_b = [sb(f"spkm{p}", [128, BL], u8) for p in range(2)]
    v1_b = [sb(f"v1{p}", [128, BL], b16) for p in range(2)]
    i0t = sb("i0t", [128, BL], f32)
    zbt = sb("zbt", [128, BL], b16)
    lnC0h_t = sb("lnC0h_t", [128, 1], f32)
    eps_t = sb("eps_t", [128, 1], f32)
    zero_t = sb("zero_t", [128, 1], f32)
    psa_b = [ps(f"psa{p}", [128, BL], f32) for p in range(2)]
    pss_b = [ps(f"pss{p}", [128, BL], f32) for p in range(2)]

    with (
        nc.Block() as block,
        nc.semaphore("x_sem") as x_sem,
        nc.semaphore("w_sem") as w_sem,
        nc.semaphore("st_sem") as st_sem,
        nc.semaphore("mm_sem") as mm_sem,
        nc.semaphore("evac_sem") as evac_sem,
        nc.semaphore("act_sem") as act_sem,
        nc.semaphore("gp_sem") as gp_sem,
        nc.semaphore("gp2_sem") as gp2_sem,
        nc.semaphore("dvs_sem") as dvs_sem,
        nc.semaphore("dv_sem") as dv_sem,
        nc.semaphore("out_sem") as out_sem,
        nc.semaphore("cst_sem") as cst_sem,
    ):
        @block.sync
        def _(sync):
            for k in range(KT):
                sync.dma_start(out=xt_sb[k].ap(),
                               in_=xt_d[k * 128:(k + 1) * 128, :]
                               ).then_inc(x_sem, 16)
            for ii in range(repeat * OT):
                i = ii % OT
                p = ii % 2
                if ii >= 2:
                    sync.wait_ge(mm_sem, ii - 1)
                sync.dma_start(
                    out=wa_b[p].ap().rearrange("r (k c) -> r k c", k=KT),
                    in_=wa_d[i].rearrange("k r c -> r k c")
                ).then_inc(w_sem, 16)
                sync.dma_start(
                    out=ws_b[p].ap().rearrange("r (k c) -> r k c", k=KT),
                    in_=ws_d[i].rearrange("k r c -> r k c")
                ).then_inc(w_sem, 16)
                if ii >= 2:
                    sync.wait_ge(dv_sem, ii - 1)
                rs = slice(i * 128, (i + 1) * 128)
                sync.dma_start(out=im0_b[p].ap(), in_=im_d[rs, :]
                               ).then_inc(st_sem, 16)
                sync.dma_start(out=ia0_b[p].ap(), in_=ia_d[rs, :]
                               ).then_inc(st_sem, 16)
                sync.dma_start(out=is0_b[p].ap(), in_=is_d[rs, :]
                               ).then_inc(st_sem, 16)
                sync.dma_start(out=rf0_b[p].ap(), in_=rf_d[rs, :]
                               ).then_inc(st_sem, 16)

        @block.tensor
        def _(pe):
            pe.wait_ge(x_sem, 16 * KT)
            for ii in range(repeat * OT):
                p = ii % 2
                pe.wait_ge(w_sem, 32 * (ii + 1))
                if ii >= 2:
                    pe.wait_ge(evac_sem, ii - 1)
                last = None
                for k in range(KT):
                    st, sp = (k == 0), (k == KT - 1)
                    pe.matmul(psa_b[p].ap(),
                              wa_b[p].ap()[:, k * 128:(k + 1) * 128],
                              xt_sb[k].ap(), start=st, stop=sp)
                    last = pe.matmul(pss_b[p].ap(),
                                     ws_b[p].ap()[:, k * 128:(k + 1) * 128],
                                     xt_sb[k].ap(), start=st, stop=sp)
                last.then_inc(mm_sem, 1)

        @block.scalar
        def _(act):
            act.wait_ge(cst_sem, 1)
            for ii in range(repeat * OT):
                i = ii % OT
                p = ii % 2
                act.wait_ge(mm_sem, ii + 1)
                act.wait_ge(st_sem, 64 * (ii + 1))
                if ii >= 2:
                    act.wait_ge(dv_sem, ii - 1)
                act.activation(T["A"][p].ap(), psa_b[p].ap(), Act.Copy,
                               scale=ca)
                act.activation(T["S"][p].ap(), pss_b[p].ap(), Act.Copy,
                               scale=cs).then_inc(evac_sem, 1)
                act.activation(T["lnim"][p].ap(), im0_b[p].ap(), Act.Ln,
                               bias=zero_t.ap())
                act.activation(T["v"][p].ap(), T["lnim"][p].ap(), Act.Exp,
                               scale=p_pow, bias=lnC0h_t.ap())
                act.activation(T["w"][p].ap(), im0_b[p].ap(), Act.Copy,
                               scale=1e12, bias=1.0)
                act.activation(T["l2"][p].ap(), im0_b[p].ap(), Act.Ln,
                               bias=eps_t.ap())
                act.activation(T["r2"][p].ap(), T["l2"][p].ap(), Act.Exp,
                               scale=-1.0, bias=zero_t.ap())
                act.activation(T["G"][p].ap(), T["r2"][p].ap(), Act.Copy,
                               scale=G_scale, bias=G_bias
                               ).then_inc(act_sem, 1)
                if ii >= 1:
                    q_ = (ii - 1) % 2
                    o_rs = slice(((ii - 1) % OT) * 128,
                                 (((ii - 1) % OT) + 1) * 128)
                    act.wait_ge(dv_sem, ii)
                    act.wait_ge(gp2_sem, ii)
                    act.dma_start(out=imem_o[o_rs, :], in_=imem_b[q_].ap()
                                  ).then_inc(out_sem, 16)
                    act.dma_start(out=iamp_o[o_rs, :], in_=iamp_b[q_].ap()
                                  ).then_inc(out_sem, 16)
                    act.dma_start(out=ishu_o[o_rs, :], in_=ishu_b[q_].ap()
                                  ).then_inc(out_sem, 16)
                    act.dma_start(out=spk_o[o_rs, :], in_=spk_b[q_].ap()
                                  ).then_inc(out_sem, 16)
                    act.dma_start(out=refr_o[o_rs, :], in_=refr_b[q_].ap()
                                  ).then_inc(out_sem, 16)
            act.wait_ge(dv_sem, repeat * OT)
            act.wait_ge(gp2_sem, repeat * OT)
            q_ = (repeat * OT - 1) % 2
            o_rs = slice((OT - 1) * 128, OT * 128)
            act.dma_start(out=imem_o[o_rs, :], in_=imem_b[q_].ap()
                          ).then_inc(out_sem, 16)
            act.dma_start(out=iamp_o[o_rs, :], in_=iamp_b[q_].ap()
                          ).then_inc(out_sem, 16)
            act.dma_start(out=ishu_o[o_rs, :], in_=ishu_b[q_].ap()
                          ).then_inc(out_sem, 16)
            act.dma_start(out=spk_o[o_rs, :], in_=spk_b[q_].ap()
                          ).then_inc(out_sem, 16)
            act.dma_start(out=refr_o[o_rs, :], in_=refr_b[q_].ap()
                          ).then_inc(out_sem, 16)

        @block.gpsimd
        def _(gp):
            for ii in range(repeat * OT):
                p = ii % 2
                gp.wait_ge(act_sem, ii + 1)
                if ii >= 2:
                    gp.wait_ge(out_sem, 80 * (ii - 1))
                gp.tensor_scalar(T["ua"][p].ap(), ia0_b[p].ap(), k1a, None,
                                 Alu.mult)
                gp.tensor_tensor(T["ua"][p].ap(), T["ua"][p].ap(),
                                 T["A"][p].ap(), Alu.add)
                gp.tensor_scalar(iamp_b[p].ap(), T["ua"][p].ap(), I0, None,
                                 Alu.max)
                gp.tensor_scalar(T["us"][p].ap(), is0_b[p].ap(), k1s, None,
                                 Alu.mult)
                gp.tensor_tensor(T["us"][p].ap(), T["us"][p].ap(),
                                 T["S"][p].ap(), Alu.add)
                gp.tensor_scalar(ishu_b[p].ap(), T["us"][p].ap(), I0, None,
                                 Alu.max)
                gp.tensor_scalar(T["m"][p].ap(), rf0_b[p].ap(), 0.0, None,
                                 Alu.is_le)
                gp.tensor_scalar(T["e"][p].ap(), ia0_b[p].ap(), c_in, None,
                                 Alu.add)
                gp.tensor_tensor(T["e"][p].ap(), T["e"][p].ap(),
                                 is0_b[p].ap(), Alu.subtract)
                gp.tensor_scalar(v1_b[p].ap(), rf0_b[p].ap(), -dt, 0.0,
                                 Alu.add, Alu.max).then_inc(gp_sem, 1)
                gp.wait_ge(dvs_sem, ii + 1)
                gp.tensor_copy(spk_b[p].ap(), spkm_b[p].ap()
                               ).then_inc(gp2_sem, 1)

        @block.vector
        def _(dv):
            dv.memset(i0t.ap(), I0)
            dv.memset(zbt.ap(), 0.0)
            dv.memset(lnC0h_t.ap(), lnC0h)
            dv.memset(eps_t.ap(), 1e-12)
            dv.memset(zero_t.ap(), 0.0).then_inc(cst_sem, 1)
            for ii in range(repeat * OT):
                p = ii % 2
                dv.wait_ge(gp_sem, ii + 1)
                if ii >= 2:
                    dv.wait_ge(out_sem, 80 * (ii - 1))
                dv.tensor_tensor(T["d1"][p].ap(), T["A"][p].ap(),
                                 T["S"][p].ap(), Alu.subtract)
                dv.tensor_tensor(T["q"][p].ap(), T["e"][p].ap(),
                                 T["d1"][p].ap(), Alu.add)
                dv.tensor_tensor(T["qm"][p].ap(), T["q"][p].ap(),
                                 T["m"][p].ap(), Alu.mult)
                dv.tensor_scalar(T["iin"][p].ap(), T["qm"][p].ap(), I0, None,
                                 Alu.max)
                dv.tensor_tensor(T["f"][p].ap(), T["v"][p].ap(),
                                 T["w"][p].ap(), Alu.mult)
                dv.scalar_tensor_tensor(T["n1"][p].ap(), im0_b[p].ap(), -1.05,
                                        T["f"][p].ap(), Alu.mult, Alu.add)
                dv.scalar_tensor_tensor(T["num"][p].ap(), T["n1"][p].ap(),
                                        -c2, T["iin"][p].ap(), Alu.add,
                                        Alu.add)
                dv.tensor_tensor(T["dimdt"][p].ap(), T["num"][p].ap(),
                                 T["G"][p].ap(), Alu.mult)
                dv.tensor_tensor(imem_b[p].ap(), im0_b[p].ap(),
                                 T["dimdt"][p].ap(), Alu.add)
                dv.tensor_scalar(imem_b[p].ap(), imem_b[p].ap(), I0, None,
                                 Alu.max)
                dv.tensor_scalar(spkm_b[p].ap(), imem_b[p].ap(), Ith, None,
                                 Alu.is_gt).then_inc(dvs_sem, 1)
                dv.copy_predicated(imem_b[p].ap(), spkm_b[p].ap(), i0t.ap())
                dv.tensor_copy(refr_b[p].ap(), v1_b[p].ap())
                dv.copy_predicated(refr_b[p].ap(), spkm_b[p].ap(), zbt.ap()
                                   ).then_inc(dv_sem, 1)

    nc.finalize()
    return nc


def _tile_w(wt):
    # [N_IN, N_OUT] -> [OT, KT, 128, 128] contiguous (ot, k, k128, o128)
    return np.ascontiguousarray(
        wt.reshape(KT, 128, OT, 128).transpose(2, 0, 1, 3))


def _prep_in_maps(X, W_ampa, W_shunt, Imem, Iampa, Ishunt, refractory):
    XT = np.ascontiguousarray(np.asarray(X, np.float32).T).astype(bf16)
    WaT = _tile_w(np.round(np.asarray(W_ampa, np.float32)).T.astype(bf16))
    WsT = _tile_w(np.round(np.asarray(W_shunt, np.float32)).T.astype(bf16))
    ImT = np.ascontiguousarray(np.asarray(Imem, np.float32).T)
    IaT = np.ascontiguousarray(np.asarray(Iampa, np.float32).T).astype(bf16)
    IsT = np.ascontiguousarray(np.asarray(Ishunt, np.float32).T).astype(bf16)
    RfT = np.ascontiguousarray(np.asarray(refractory, np.float32).T
                               ).astype(bf16)

    in_maps = []
    for c in range(NCORES):
        cs_ = slice(c * BL, (c + 1) * BL)
        in_maps.append(dict(
            xt=np.ascontiguousarray(XT[:, cs_]),
            wat=WaT, wst=WsT,
            im0=np.ascontiguousarray(ImT[:, cs_]),
            ia0=np.ascontiguousarray(IaT[:, cs_]),
            is0=np.ascontiguousarray(IsT[:, cs_]),
            rf0=np.ascontiguousarray(RfT[:, cs_]),
        ))
    return in_maps


def kernel(X, W_ampa, W_shunt, Iw_ampa, Iw_shunt, Imem, Iampa, Ishunt,
           refractory):
    from concourse.bass_utils import run_bass_kernel_spmd

    ca = float(np.float32(Iw_ampa))          # Igain_ampa/Itau_ampa == 1
    cs = float(np.float32(Iw_shunt))
    key = (ca, cs)
    if key not in _CACHE:
        _CACHE[key] = _build(ca, cs)
    nc = _CACHE[key]

    in_maps = _prep_in_maps(X, W_ampa, W_shunt, Imem, Iampa, Ishunt,
                            refractory)
    global LAST_RESULT
    r = run_bass_kernel_spmd(nc, in_maps, list(range(NCORES)))
    LAST_RESULT = r
    res = r.results

    def gather(name, cast=False):
        full = np.concatenate([r[name] for r in res], 1)   # [N_OUT, B]
        out = np.ascontiguousarray(full.T)
        return out.astype(np.float32) if cast else out

    spike = gather("spk", cast=True)
    imem = gather("imem")
    iampa = gather("iamp")
    ishunt = gather("ishu")
    refr = gather("refr", cast=True)
    return spike, (imem, iampa, ishunt, refr)


# revision 17
# speedup vs baseline: 446.2877x; 97.0568x over previous
"""DPI neuron forward step on 8 Trainium2 cores (raw Bass, explicit sems).

Math (forward only, f32):
  Sa = X @ round(W_ampa).T ; Ss = X @ round(W_shunt).T
  A = Iw_a*Sa ; S = Iw_s*Ss
  Iin  = max((Ia0 + A - Is0 - S + c_in) * (rf0<=0), I0)
  Ifb  = (C0/2) * Im0^p          (sigmoid denom == 2.0 exactly in f32)
  f    = Ifb * (1e12*Im0 + 1)
  num  = Iin - c2 - 1.05*Im0 + f
  G    = (dt/taum) * Im0/(Im0+1e-12)
  Im1  = max(Im0 + num*G, I0)
  spike = Im1 > Ith ; Imem = spike? I0 : Im1
  Iampa = max(k1*Ia0 + A, I0) ; Ishunt = max(k1*Is0 + S, I0)
  refr  = spike? 0 : max(rf0-dt, 0)

Sharding: batch 4096 -> 8 cores x 512 rows; weights replicated (bf16 is
exact for round(W) in 0..4). On-device layout is TRANSPOSED [n_out, b]:
psum[o=128, b=512] = W_tile[k,o].T @ XT[k,b], W tiles streamed as the
stationary operand (host pre-tiled contiguous), X.T resident moving.
16 o_tile iterations, 2-deep double-buffered pipeline across
SP(dma-in) / PE / ACT(+dma-out) / GPSIMD / DVE with explicit semaphores.
"""

import math
import numpy as np
import ml_dtypes

bf16 = ml_dtypes.bfloat16

B, N_IN, N_OUT = 4096, 2048, 2048
NCORES = 8
BL = B // NCORES          # 512 batch rows per core
OT = N_OUT // 128         # 16 o_tiles
KT = N_IN // 128          # 16 contraction tiles

# model constants
I0 = 5e-14
kappa = (0.75 + 0.66) / 2
dt = 1e-3
tau_mem = 0.025 / kappa * 3.0
tau_ampa = 0.025 / kappa * 2.0
tau_shunt = 0.025 / kappa * 2.0
Ith = 1e-12
p_pow = kappa / (kappa + 1.0)
lnC0h = (1.0 / (kappa + 1.0)) * math.log(I0) - math.log(2.0)
c_in = 1e-12 + I0
c2 = 1e-12 + I0
k1a = 1.0 - dt / tau_ampa
k1s = 1.0 - dt / tau_shunt
G_scale = -(1e-12) * dt / tau_mem
G_bias = dt / tau_mem

_CACHE = {}
LAST_RESULT = None


def _build(ca: float, cs: float, repeat: int = 1, mode: str = "full"):
    inc_pe = mode in ("full", "gemm")
    inc_el = mode in ("full", "elem")
    inc_st = mode in ("full", "elem", "dma")
    inc_out = mode in ("full", "elem", "dma")
    import concourse.bass as bass
    from concourse import mybir

    f32 = mybir.dt.float32
    b16 = mybir.dt.bfloat16
    u8 = mybir.dt.uint8
    Alu = mybir.AluOpType
    Act = mybir.ActivationFunctionType

    nc = bass.Bass()
    xt_d = nc.declare_dram_parameter("xt", [N_IN, BL], b16, isOutput=False)
    wa_d = nc.declare_dram_parameter("wat", [OT, KT, 128, 128], b16,
                                     isOutput=False)
    ws_d = nc.declare_dram_parameter("wst", [OT, KT, 128, 128], b16,
                                     isOutput=False)
    im_d = nc.declare_dram_parameter("im0", [N_OUT, BL], f32, isOutput=False)
    ia_d = nc.declare_dram_parameter("ia0", [N_OUT, BL], b16, isOutput=False)
    is_d = nc.declare_dram_parameter("is0", [N_OUT, BL], b16, isOutput=False)
    rf_d = nc.declare_dram_parameter("rf0", [N_OUT, BL], b16, isOutput=False)
    spk_o = nc.declare_dram_parameter("spk", [N_OUT, BL], b16, isOutput=True)
    imem_o = nc.declare_dram_parameter("imem", [N_OUT, BL], f32, isOutput=True)
    iamp_o = nc.declare_dram_parameter("iamp", [N_OUT, BL], f32, isOutput=True)
    ishu_o = nc.declare_dram_parameter("ishu", [N_OUT, BL], f32, isOutput=True)
    refr_o = nc.declare_dram_parameter("refr", [N_OUT, BL], b16, isOutput=True)

    sb = nc.alloc_sbuf_tensor
    ps = nc.alloc_psum_tensor
    xt_sb = [sb(f"xt{k}", [128, BL], b16) for k in range(KT)]
    wa_b = [sb(f"wa{p}", [128, KT * 128], b16) for p in range(2)]
    ws_b = [sb(f"ws{p}", [128, KT * 128], b16) for p in range(2)]
    im0_b = [sb(f"im0{p}", [128, BL], f32) for p in range(2)]
    ia0_b = [sb(f"ia0{p}", [128, BL], b16) for p in range(2)]
    is0_b = [sb(f"is0{p}", [128, BL], b16) for p in range(2)]
    rf0_b = [sb(f"rf0{p}", [128, BL], b16) for p in range(2)]
    imem_b = [sb(f"imem{p}", [128, BL], f32) for p in range(2)]
    iamp_b = [sb(f"iamp{p}", [128, BL], f32) for p in range(2)]
    ishu_b = [sb(f"ishu{p}", [128, BL], f32) for p in range(2)]
    spk_b = [sb(f"spk{p}", [128, BL], b16) for p in range(2)]
    refr_b = [sb(f"refr{p}", [128, BL], b16) for p in range(2)]
    names = "A S lnim v w l2 r2 G ua us m e d1 q qm iin f n1 num dimdt".split()
    T = {nm: [sb(f"{nm}{p}", [128, BL], f32) for p in range(2)] for nm in names}
    spkm_b = [sb(f"spkm{p}", [128, BL], u8) for p in range(2)]
    v1_b = [sb(f"v1{p}", [128, BL], b16) for p in range(2)]
    i0t = sb("i0t", [128, BL], f32)
    zbt = sb("zbt", [128, BL], b16)
    lnC0h_t = sb("lnC0h_t", [128, 1], f32)
    eps_t = sb("eps_t", [128, 1], f32)
    zero_t = sb("zero_t", [128, 1], f32)
    psa_b = [ps(f"psa{p}", [128, BL], f32) for p in range(2)]
    pss_b = [ps(f"pss{p}", [128, BL], f32) for p in range(2)]

    with (
        nc.Block() as block,
        nc.semaphore("x_sem") as x_sem,
        nc.semaphore("w_sem") as w_sem,
        nc.semaphore("st_sem") as st_sem,
        nc.semaphore("mm_sem") as mm_sem,
        nc.semaphore("evac_sem") as evac_sem,
        nc.semaphore("act_sem") as act_sem,
        nc.semaphore("gp_sem") as gp_sem,
        nc.semaphore("gp2_sem") as gp2_sem,
        nc.semaphore("dvs_sem") as dvs_sem,
        nc.semaphore("dv_sem") as dv_sem,
        nc.semaphore("out_sem") as out_sem,
        nc.semaphore("cst_sem") as cst_sem,
    ):
        @block.sync
        def _(sync):
            for k in range(KT):
                sync.dma_start(out=xt_sb[k].ap(),
                               in_=xt_d[k * 128:(k + 1) * 128, :]
                               ).then_inc(x_sem, 16)
            for ii in range(repeat * OT):
                i = ii % OT
                p = ii % 2
                if inc_pe or mode == "dma":
                    if ii >= 2 and inc_pe:
                        sync.wait_ge(mm_sem, ii - 1)
                    sync.dma_start(
                        out=wa_b[p].ap().rearrange("r (k c) -> r k c", k=KT),
                        in_=wa_d[i].rearrange("k r c -> r k c")
                    ).then_inc(w_sem, 16)
                    sync.dma_start(
                        out=ws_b[p].ap().rearrange("r (k c) -> r k c", k=KT),
                        in_=ws_d[i].rearrange("k r c -> r k c")
                    ).then_inc(w_sem, 16)
                if inc_st:
                    if ii >= 2 and inc_el:
                        sync.wait_ge(dv_sem, ii - 1)
                    rs = slice(i * 128, (i + 1) * 128)
                    sync.dma_start(out=im0_b[p].ap(), in_=im_d[rs, :]
                                   ).then_inc(st_sem, 16)
                    sync.dma_start(out=ia0_b[p].ap(), in_=ia_d[rs, :]
                                   ).then_inc(st_sem, 16)
                    sync.dma_start(out=is0_b[p].ap(), in_=is_d[rs, :]
                                   ).then_inc(st_sem, 16)
                    sync.dma_start(out=rf0_b[p].ap(), in_=rf_d[rs, :]
                                   ).then_inc(st_sem, 16)

        @block.tensor
        def _(pe):
            if not inc_pe:
                return
            pe.wait_ge(x_sem, 16 * KT)
            for ii in range(repeat * OT):
                p = ii % 2
                pe.wait_ge(w_sem, 32 * (ii + 1))
                if ii >= 2 and inc_el:
                    pe.wait_ge(evac_sem, ii - 1)
                last = None
                for k in range(KT):
                    st, sp = (k == 0), (k == KT - 1)
                    pe.matmul(psa_b[p].ap(),
                              wa_b[p].ap()[:, k * 128:(k + 1) * 128],
                              xt_sb[k].ap(), start=st, stop=sp)
                    last = pe.matmul(pss_b[p].ap(),
                                     ws_b[p].ap()[:, k * 128:(k + 1) * 128],
                                     xt_sb[k].ap(), start=st, stop=sp)
                last.then_inc(mm_sem, 1)

        @block.scalar
        def _(act):
            act.wait_ge(cst_sem, 1)
            for ii in range(repeat * OT):
                i = ii % OT
                p = ii % 2
                if not inc_el:
                    break
                if inc_pe:
                    act.wait_ge(mm_sem, ii + 1)
                act.wait_ge(st_sem, 64 * (ii + 1))
                if ii >= 2:
                    act.wait_ge(dv_sem, ii - 1)
                act.activation(T["A"][p].ap(), psa_b[p].ap(), Act.Copy,
                               scale=ca)
                act.activation(T["S"][p].ap(), pss_b[p].ap(), Act.Copy,
                               scale=cs).then_inc(evac_sem, 1)
                act.activation(T["lnim"][p].ap(), im0_b[p].ap(), Act.Ln,
                               bias=zero_t.ap())
                act.activation(T["v"][p].ap(), T["lnim"][p].ap(), Act.Exp,
                               scale=p_pow, bias=lnC0h_t.ap())
                act.activation(T["w"][p].ap(), im0_b[p].ap(), Act.Copy,
                               scale=1e12, bias=1.0)
                act.activation(T["l2"][p].ap(), im0_b[p].ap(), Act.Ln,
                               bias=eps_t.ap())
                act.activation(T["r2"][p].ap(), T["l2"][p].ap(), Act.Exp,
                               scale=-1.0, bias=zero_t.ap())
                act.activation(T["G"][p].ap(), T["r2"][p].ap(), Act.Copy,
                               scale=G_scale, bias=G_bias
                               ).then_inc(act_sem, 1)
                if ii >= 1:
                    q_ = (ii - 1) % 2
                    o_rs = slice(((ii - 1) % OT) * 128,
                                 (((ii - 1) % OT) + 1) * 128)
                    act.wait_ge(dv_sem, ii)
                    act.wait_ge(gp2_sem, ii)
                    act.dma_start(out=imem_o[o_rs, :], in_=imem_b[q_].ap()
                                  ).then_inc(out_sem, 16)
                    act.dma_start(out=iamp_o[o_rs, :], in_=iamp_b[q_].ap()
                                  ).then_inc(out_sem, 16)
                    act.dma_start(out=ishu_o[o_rs, :], in_=ishu_b[q_].ap()
                                  ).then_inc(out_sem, 16)
                    act.dma_start(out=spk_o[o_rs, :], in_=spk_b[q_].ap()
                                  ).then_inc(out_sem, 16)
                    act.dma_start(out=refr_o[o_rs, :], in_=refr_b[q_].ap()
                                  ).then_inc(out_sem, 16)
            if mode == "dma":
                for ii in range(repeat * OT):
                    i = ii % OT
                    q2 = ii % 2
                    o_rs2 = slice(i * 128, (i + 1) * 128)
                    act.dma_start(out=imem_o[o_rs2, :], in_=imem_b[q2].ap()
                                  ).then_inc(out_sem, 16)
                    act.dma_start(out=iamp_o[o_rs2, :], in_=iamp_b[q2].ap()
                                  ).then_inc(out_sem, 16)
                    act.dma_start(out=ishu_o[o_rs2, :], in_=ishu_b[q2].ap()
                                  ).then_inc(out_sem, 16)
                    act.dma_start(out=spk_o[o_rs2, :], in_=spk_b[q2].ap()
                                  ).then_inc(out_sem, 16)
                    act.dma_start(out=refr_o[o_rs2, :], in_=refr_b[q2].ap()
                                  ).then_inc(out_sem, 16)
            if not inc_el:
                return
            act.wait_ge(dv_sem, repeat * OT)
            act.wait_ge(gp2_sem, repeat * OT)
            q_ = (repeat * OT - 1) % 2
            o_rs = slice((OT - 1) * 128, OT * 128)
            act.dma_start(out=imem_o[o_rs, :], in_=imem_b[q_].ap()
                          ).then_inc(out_sem, 16)
            act.dma_start(out=iamp_o[o_rs, :], in_=iamp_b[q_].ap()
                          ).then_inc(out_sem, 16)
            act.dma_start(out=ishu_o[o_rs, :], in_=ishu_b[q_].ap()
                          ).then_inc(out_sem, 16)
            act.dma_start(out=spk_o[o_rs, :], in_=spk_b[q_].ap()
                          ).then_inc(out_sem, 16)
            act.dma_start(out=refr_o[o_rs, :], in_=refr_b[q_].ap()
                          ).then_inc(out_sem, 16)

        @block.gpsimd
        def _(gp):
            if not inc_el:
                return
            for ii in range(repeat * OT):
                p = ii % 2
                gp.wait_ge(act_sem, ii + 1)
                if ii >= 2:
                    gp.wait_ge(out_sem, 80 * (ii - 1))
                gp.tensor_scalar(T["ua"][p].ap(), ia0_b[p].ap(), k1a, None,
                                 Alu.mult)
                gp.tensor_tensor(T["ua"][p].ap(), T["ua"][p].ap(),
                                 T["A"][p].ap(), Alu.add)
                gp.tensor_scalar(iamp_b[p].ap(), T["ua"][p].ap(), I0, None,
                                 Alu.max)
                gp.tensor_scalar(T["us"][p].ap(), is0_b[p].ap(), k1s, None,
                                 Alu.mult)
                gp.tensor_tensor(T["us"][p].ap(), T["us"][p].ap(),
                                 T["S"][p].ap(), Alu.add)
                gp.tensor_scalar(ishu_b[p].ap(), T["us"][p].ap(), I0, None,
                                 Alu.max)
                gp.tensor_scalar(T["m"][p].ap(), rf0_b[p].ap(), 0.0, None,
                                 Alu.is_le)
                gp.tensor_scalar(T["e"][p].ap(), ia0_b[p].ap(), c_in, None,
                                 Alu.add)
                gp.tensor_tensor(T["e"][p].ap(), T["e"][p].ap(),
                                 is0_b[p].ap(), Alu.subtract)
                gp.tensor_scalar(v1_b[p].ap(), rf0_b[p].ap(), -dt, 0.0,
                                 Alu.add, Alu.max).then_inc(gp_sem, 1)
                gp.wait_ge(dvs_sem, ii + 1)
                gp.tensor_copy(spk_b[p].ap(), spkm_b[p].ap()
                               ).then_inc(gp2_sem, 1)

        @block.vector
        def _(dv):
            dv.memset(i0t.ap(), I0)
            dv.memset(zbt.ap(), 0.0)
            dv.memset(lnC0h_t.ap(), lnC0h)
            dv.memset(eps_t.ap(), 1e-12)
            dv.memset(zero_t.ap(), 0.0).then_inc(cst_sem, 1)
            if not inc_el:
                return
            for ii in range(repeat * OT):
                p = ii % 2
                dv.wait_ge(gp_sem, ii + 1)
                if ii >= 2:
                    dv.wait_ge(out_sem, 80 * (ii - 1))
                dv.tensor_tensor(T["d1"][p].ap(), T["A"][p].ap(),
                                 T["S"][p].ap(), Alu.subtract)
                dv.tensor_tensor(T["q"][p].ap(), T["e"][p].ap(),
                                 T["d1"][p].ap(), Alu.add)
                dv.tensor_tensor(T["qm"][p].ap(), T["q"][p].ap(),
                                 T["m"][p].ap(), Alu.mult)
                dv.tensor_scalar(T["iin"][p].ap(), T["qm"][p].ap(), I0, None,
                                 Alu.max)
                dv.tensor_tensor(T["f"][p].ap(), T["v"][p].ap(),
                                 T["w"][p].ap(), Alu.mult)
                dv.scalar_tensor_tensor(T["n1"][p].ap(), im0_b[p].ap(), -1.05,
                                        T["f"][p].ap(), Alu.mult, Alu.add)
                dv.scalar_tensor_tensor(T["num"][p].ap(), T["n1"][p].ap(),
                                        -c2, T["iin"][p].ap(), Alu.add,
                                        Alu.add)
                dv.tensor_tensor(T["dimdt"][p].ap(), T["num"][p].ap(),
                                 T["G"][p].ap(), Alu.mult)
                dv.tensor_tensor(imem_b[p].ap(), im0_b[p].ap(),
                                 T["dimdt"][p].ap(), Alu.add)
                dv.tensor_scalar(imem_b[p].ap(), imem_b[p].ap(), I0, None,
                                 Alu.max)
                dv.tensor_scalar(spkm_b[p].ap(), imem_b[p].ap(), Ith, None,
                                 Alu.is_gt).then_inc(dvs_sem, 1)
                dv.copy_predicated(imem_b[p].ap(), spkm_b[p].ap(), i0t.ap())
                dv.tensor_copy(refr_b[p].ap(), v1_b[p].ap())
                dv.copy_predicated(refr_b[p].ap(), spkm_b[p].ap(), zbt.ap()
                                   ).then_inc(dv_sem, 1)

    nc.finalize()
    return nc


def _tile_w(wt):
    # [N_IN, N_OUT] -> [OT, KT, 128, 128] contiguous (ot, k, k128, o128)
    return np.ascontiguousarray(
        wt.reshape(KT, 128, OT, 128).transpose(2, 0, 1, 3))


def _prep_in_maps(X, W_ampa, W_shunt, Imem, Iampa, Ishunt, refractory):
    XT = np.ascontiguousarray(np.asarray(X, np.float32).T).astype(bf16)
    WaT = _tile_w(np.round(np.asarray(W_ampa, np.float32)).T.astype(bf16))
    WsT = _tile_w(np.round(np.asarray(W_shunt, np.float32)).T.astype(bf16))
    ImT = np.ascontiguousarray(np.asarray(Imem, np.float32).T)
    IaT = np.ascontiguousarray(np.asarray(Iampa, np.float32).T).astype(bf16)
    IsT = np.ascontiguousarray(np.asarray(Ishunt, np.float32).T).astype(bf16)
    RfT = np.ascontiguousarray(np.asarray(refractory, np.float32).T
                               ).astype(bf16)

    in_maps = []
    for c in range(NCORES):
        cs_ = slice(c * BL, (c + 1) * BL)
        in_maps.append(dict(
            xt=np.ascontiguousarray(XT[:, cs_]),
            wat=WaT, wst=WsT,
            im0=np.ascontiguousarray(ImT[:, cs_]),
            ia0=np.ascontiguousarray(IaT[:, cs_]),
            is0=np.ascontiguousarray(IsT[:, cs_]),
            rf0=np.ascontiguousarray(RfT[:, cs_]),
        ))
    return in_maps


def kernel(X, W_ampa, W_shunt, Iw_ampa, Iw_shunt, Imem, Iampa, Ishunt,
           refractory):
    from concourse.bass_utils import run_bass_kernel_spmd

    ca = float(np.float32(Iw_ampa))          # Igain_ampa/Itau_ampa == 1
    cs = float(np.float32(Iw_shunt))
    key = (ca, cs)
    if key not in _CACHE:
        _CACHE[key] = _build(ca, cs)
    nc = _CACHE[key]

    in_maps = _prep_in_maps(X, W_ampa, W_shunt, Imem, Iampa, Ishunt,
                            refractory)
    global LAST_RESULT
    r = run_bass_kernel_spmd(nc, in_maps, list(range(NCORES)))
    LAST_RESULT = r
    res = r.results

    def gather(name, cast=False):
        full = np.concatenate([r[name] for r in res], 1)   # [N_OUT, B]
        out = np.ascontiguousarray(full.T)
        return out.astype(np.float32) if cast else out

    spike = gather("spk", cast=True)
    imem = gather("imem")
    iampa = gather("iamp")
    ishunt = gather("ishu")
    refr = gather("refr", cast=True)
    return spike, (imem, iampa, ishunt, refr)
